# revision 20
# baseline (speedup 1.0000x reference)
"""CategoryAttention (softmax over heads axis) on 8 Trainium2 cores.

Sharding: B*L = 2*2048 = 4096 query rows split 8 ways (512 rows/core).
Core c handles batch b=c//4, query rows [(c%4)*512, (c%4+1)*512).
The softmax is over the 16 heads, which is fully local to each (q,k)
position, so no cross-core communication is needed. Each core
recomputes K/V projections for its batch (4x redundant).

Per-core pipeline (layouts transposed so the model dim rides the SBUF
partition axis; all big matmuls in float32r at full PE rate):
  phase 1: Q^T = Wq^T.T @ q^T;  K^T = Wk^T.T @ k^T;  V = v^T.T @ Wv^T
  phase 2: per k-tile (q = all 512 rows at once):
    e_h^T[k,q] = Kh^T.T @ Qh^T   (16 heads; 2-head psum tiles x3 so
                                  the PE/ACT exp pipeline overlaps)
    p_h = exp(e_h/8)             (ACT, psum->sbuf, bf16)
    den = sum_h p_h              (DVE bf16 tree, f32 final add)
    attn = p * (1/den)           (DVE approx-recip + bcast mult at 2x,
                                  a slice of heads on GPSIMD)
    ctx_h^T += Vh.T @ attn_h^T   (PSUM accum over KC=2 k-tiles,
                                  1 full bank per head-pair, DVE drain)
  phase 3: out^T = Wo^T.T @ ctx^T + bias
"""

import os

import numpy as np
from contextlib import ExitStack

import concourse.bass as bass
import concourse.tile as tile
from concourse import bacc, mybir
from concourse.bass_utils import run_bass_kernel_spmd

F32 = mybir.dt.float32
F32R = mybir.dt.float32r
BF16 = mybir.dt.bfloat16

N_CORES = 8
P = 128
D = 1024          # d_model
S = D // P        # 8 subtiles of the contraction dim
H = 16            # heads
HD = 64           # head dim
B = 2
L = 2048
LQ = L * B // N_CORES   # 512 query rows per core
LK = L                  # key rows per core (full batch slice)
KTS = 128               # k tile
NKT = LK // KTS         # 16
KC = 2                  # k-tiles per AV psum accumulation chunk
KC_F = 4                # k-tiles per chunk in the flipped-AV path
GP_HEADS = int(os.environ.get("GP_HEADS", "0"))  # heads normalized on GPSIMD
SCALE = 1.0 / np.sqrt(HD)

ATT_DT = BF16           # exp/attn planes and V dtype
MM_DT = BF16            # all matmul operands (f32 psum accumulate)

BENCH_LOOP = int(os.environ.get("BENCH_LOOP", "1"))
AV_FLIP = int(os.environ.get("AV_FLIP", "0"))  # attn-as-weights AV path


def _build(has_bias):
    nc = bacc.Bacc("TRN2", target_bir_lowering=False, debug=False, num_devices=1)

    def din(name, shape, dt=F32):
        return nc.dram_tensor(name, shape, dt, kind="ExternalInput").ap()

    qT_d = din("qT", (P, S * LQ), MM_DT)
    kT_d = din("kT", (P, S * LK), MM_DT)
    vT_d = din("vT", (P, S * LK), MM_DT)
    wq_d = din("wq", (P, S * D), MM_DT)
    wk_d = din("wk", (P, S * D), MM_DT)
    wv_d = din("wv", (P, S * D), MM_DT)
    wo_d = din("wo", (P, S * D), MM_DT)
    bias_d = {}
    for nm in ("bq", "bk", "bv", "bo"):
        if has_bias[nm]:
            bias_d[nm] = din(nm, (1, D))
    ident_d = din("ident", (P, P), MM_DT) if AV_FLIP else None
    outT_d = nc.dram_tensor("outT", (P, S * LQ), F32, kind="ExternalOutput").ap()

    qT_ap = qT_d.rearrange("p (s q) -> p s q", s=S)
    kT_ap = kT_d.rearrange("p (c s k) -> p c s k", c=4, s=S)
    vT_ap = vT_d.rearrange("p (c s k) -> p c s k", c=4, s=S)
    wq_ap = wq_d.rearrange("p (h s o) -> p h s o", h=2, s=S)
    wk_ap = wk_d.rearrange("p (h s o) -> p h s o", h=2, s=S)
    wv_ap = wv_d.rearrange("p (h s o) -> p h s o", h=2, s=S)
    wo_ap = wo_d.rearrange("p (h s o) -> p h s o", h=2, s=S)
    outT_ap = outT_d.rearrange("p (j q) -> p j q", j=S)

    with tile.TileContext(nc) as tc, ExitStack() as ctx:
        if BENCH_LOOP > 1:
            ctx.enter_context(tc.For_i(0, BENCH_LOOP, 1))

        const_pool = ctx.enter_context(tc.tile_pool(name="const", bufs=1))
        qt_pool = ctx.enter_context(tc.tile_pool(name="QT", bufs=1))
        kt_pool = ctx.enter_context(tc.tile_pool(name="KT", bufs=1))
        v_pool = ctx.enter_context(tc.tile_pool(name="V", bufs=1))

        any_bias = any(has_bias.values())
        ones_t = None
        if any_bias:
            ones_t = const_pool.tile([1, 512], F32, tag="ones")
            nc.vector.memset(ones_t[:], 1.0)
        bias_t = {}
        for nm, d_ap in bias_d.items():
            t = const_pool.tile([1, D], F32, tag=f"bias_{nm}")
            nc.sync.dma_start(t[:], d_ap)
            bias_t[nm] = t

        QT_sb = qt_pool.tile([P, S, LQ], MM_DT)
        KT_sb = kt_pool.tile([P, S, LK], MM_DT)
        V_sb = v_pool.tile([P, NKT, D], ATT_DT)

        def bias_mm(ps_t, bias_name, o0, n_sz, o_on_partitions):
            if o_on_partitions:
                nc.tensor.matmul(ps_t, lhsT=bias_t[bias_name][0:1, o0:o0 + P],
                                 rhs=ones_t[0:1, :n_sz], start=False, stop=True)
            else:
                nc.tensor.matmul(ps_t, lhsT=ones_t[0:1, 0:P],
                                 rhs=bias_t[bias_name][0:1, o0:o0 + n_sz],
                                 start=False, stop=True)

        # ---------------- phase 1: Q, K, V projections ----------------
        with tc.tile_pool(name="stream", bufs=5) as spool, \
             tc.tile_pool(name="ppsum", bufs=2, space="PSUM") as ppsum:

            def stream_tile(src_ap):
                t = spool.tile([P, S, 512], MM_DT, tag="w")
                nc.sync.dma_start(t[:], src_ap)
                return t

            def proj_cols(w_halves, in_t, out_view, bias_name, n_sz):
                # out^T[o,n] = sum_i W^T[i,o]*in^T[i,n]; 4 o-tiles per psum
                # tile (one bank per matmul target), one ACT copy out.
                for j4 in range(2):
                    ps = ppsum.tile([P, 4, 512], F32, tag="pp")
                    for jl in range(4):
                        j = j4 * 4 + jl
                        w_t = w_halves[j // 4]
                        for s in range(S):
                            nc.tensor.matmul(
                                ps[:, jl, :n_sz],
                                lhsT=w_t[:, s, jl * P:(jl + 1) * P],
                                rhs=in_t[:, s, :n_sz],
                                start=(s == 0),
                                stop=(s == S - 1 and bias_name is None),
                            )
                        if bias_name is not None:
                            bias_mm(ps[:, jl, :n_sz], bias_name, j * P, n_sz,
                                    True)
                    nc.scalar.copy(out_view[:, j4 * 4:(j4 + 1) * 4, :n_sz],
                                   ps[:, :, :n_sz])

            qin = stream_tile(qT_ap)
            wq_h = [stream_tile(wq_ap[:, wh]) for wh in range(2)]
            proj_cols(wq_h, qin, QT_sb, "bq" if has_bias["bq"] else None, LQ)

            wk_h = [stream_tile(wk_ap[:, wh]) for wh in range(2)]
            for kn in range(4):
                kin = stream_tile(kT_ap[:, kn])
                proj_cols(wk_h, kin, KT_sb[:, :, kn * 512:(kn + 1) * 512],
                          "bk" if has_bias["bk"] else None, 512)

            wv_h = [stream_tile(wv_ap[:, wh]) for wh in range(2)]
            for kn in range(4):
                vin = stream_tile(vT_ap[:, kn])
                for kt4 in range(4):
                    kt = kn * 4 + kt4
                    ps = ppsum.tile([P, 4, 512], F32, tag="pp")
                    for t in range(2):  # o halves; 2 targets used of 4
                        for s in range(S):
                            nc.tensor.matmul(
                                ps[:, t, :],
                                lhsT=vin[:, s, kt4 * P:(kt4 + 1) * P],
                                rhs=wv_h[t][:, s, :],
                                start=(s == 0),
                                stop=(s == S - 1 and not has_bias["bv"]),
                            )
                        if has_bias["bv"]:
                            bias_mm(ps[:, t, :], "bv", t * 512, 512, False)
                    nc.scalar.copy(V_sb[:, kt, :],
                                   ps[:, 0:2, :].rearrange("p a b -> p (a b)"))

        # ---------------- phase 2: attention (q = 512) ----------------
        attn_pool = ctx.enter_context(
            tc.tile_pool(name="attn", bufs=5 if AV_FLIP else 3))
        tree_pool = ctx.enter_context(tc.tile_pool(name="tree", bufs=1))
        den_pool = ctx.enter_context(tc.tile_pool(name="den", bufs=1))
        r_pool = ctx.enter_context(tc.tile_pool(name="r", bufs=1))
        rb_pool = ctx.enter_context(tc.tile_pool(name="rb", bufs=1))
        ctx_pool = ctx.enter_context(tc.tile_pool(name="ctx", bufs=1))
        osb_pool = ctx.enter_context(tc.tile_pool(name="osb", bufs=1))
        wo_pool = ctx.enter_context(tc.tile_pool(name="wo", bufs=1))
        e_psum = ctx.enter_context(
            tc.tile_pool(name="epsum", bufs=2 if AV_FLIP else 3, space="PSUM"))
        av_psum = ctx.enter_context(
            tc.tile_pool(name="avpsum", bufs=1, space="PSUM"))
        if AV_FLIP:
            tp_psum = ctx.enter_context(
                tc.tile_pool(name="tppsum", bufs=2, space="PSUM"))

        # f32 ctx accumulator + bf16 copy for the O-proj rhs
        ctx_sb = ctx_pool.tile([P, S, LQ], MM_DT, tag="ctxbf")
        if AV_FLIP:
            # (qt, head, hd) layout: d-blocks of 2 heads contiguous per qt
            ctxq32 = ctx_pool.tile([P, 4, H, HD], F32, tag="cq32")
            ctxq_bf = ctx_pool.tile([P, 4, H, HD], MM_DT, tag="cqbf")
            ident_t = const_pool.tile([P, P], MM_DT, tag="ident")
            nc.sync.dma_start(ident_t[:], ident_d)
        else:
            ctx32_sb = ctx_pool.tile([P, S, LQ], F32)

        def softmax_kt(kt):
            """Energy (16 heads) -> exp -> den -> normalized attn tile."""
            attn_t = attn_pool.tile([P, H, LQ], ATT_DT, tag="attn")
            for g in range(8):  # 2 heads per psum tile, one bank per head
                eps = e_psum.tile([P, 2, LQ], F32, tag="e")
                for hh in range(2):
                    h = g * 2 + hh
                    j2, p0 = h // 2, HD * (h % 2)
                    nc.tensor.matmul(
                        eps[:, hh, :],
                        lhsT=KT_sb[p0:p0 + HD, j2, kt * KTS:(kt + 1) * KTS],
                        rhs=QT_sb[p0:p0 + HD, j2, :],
                        start=True,
                        stop=True,
                    )
                nc.scalar.activation(attn_t[:, g * 2:(g + 1) * 2, :], eps[:],
                                     mybir.ActivationFunctionType.Exp,
                                     scale=float(SCALE))
            # den = sum over heads (bf16 tree at DVE 2x; tail on GPSIMD)
            t1 = tree_pool.tile([P, 4, LQ], ATT_DT)
            with nc.allow_low_precision(reason="bf16 head-sum tree"):
                nc.vector.tensor_add(t1[:], attn_t[:, 0:4, :], attn_t[:, 4:8, :])
                nc.vector.tensor_add(t1[:], t1[:], attn_t[:, 8:12, :])
                nc.vector.tensor_add(t1[:], t1[:], attn_t[:, 12:16, :])
                nc.vector.tensor_add(t1[:, 0:2, :], t1[:, 0:2, :], t1[:, 2:4, :])
            den = den_pool.tile([P, LQ], F32)
            nc.vector.tensor_add(den[:], t1[:, 0, :], t1[:, 1, :])
            r32 = r_pool.tile([P, LQ], F32)
            nc.vector.reciprocal_approx_fast(r32[:], den[:])
            rb = rb_pool.tile([P, LQ], ATT_DT)
            nc.gpsimd.tensor_copy(rb[:], r32[:])
            nd = H - GP_HEADS
            nc.vector.tensor_mul(
                attn_t[:, 0:nd, :], attn_t[:, 0:nd, :],
                rb[:, None, :].to_broadcast((P, nd, LQ)))
            if GP_HEADS:
                nc.gpsimd.tensor_mul(
                    attn_t[:, nd:H, :], attn_t[:, nd:H, :],
                    rb[:, None, :].to_broadcast((P, GP_HEADS, LQ)))
            return attn_t

        def av_group(u, c0, attn_list, first, last):
            """One avp tile: heads 4u..4u+3, full q, over KC k-tiles."""
            avp = av_psum.tile([P, 2, LQ], F32, tag="av")
            for ci in range(KC):
                kt = c0 + ci
                for hh in range(4):
                    h = 4 * u + hh
                    i, p0 = hh // 2, HD * (hh % 2)
                    nc.tensor.matmul(
                        avp[p0:p0 + HD, i, :],
                        lhsT=V_sb[:, kt, h * HD:(h + 1) * HD],
                        rhs=attn_list[ci][:, h, :],
                        start=(ci == 0),
                        stop=(ci == KC - 1),
                    )
            sl = slice(2 * u, 2 * u + 2)
            if first:
                nc.vector.tensor_copy(ctx32_sb[:, sl, :], avp[:, :, :])
            elif last:
                with nc.allow_low_precision(reason="final ctx to bf16"):
                    nc.vector.tensor_add(ctx_sb[:, sl, :], ctx32_sb[:, sl, :],
                                         avp[:, :, :])
            else:
                nc.vector.tensor_add(ctx32_sb[:, sl, :], ctx32_sb[:, sl, :],
                                     avp[:, :, :])

        def av_flip_group(g, c0, attn_list, first, last):
            """Flipped AV: attn tiles are PE weights; heads 4g..4g+3.

            out psum [128 q, hh, qt, 64hd] accumulated over the chunk's
            KC_F k-tiles; ctx kept in [q-part, head, qt, hd] layout."""
            avp = av_psum.tile([P, 4, 4, HD], F32, tag="av")  # (qt, hh, hd)
            for hh in range(4):
                h = 4 * g + hh
                for qt in range(4):
                    for ci in range(KC_F):
                        nc.tensor.matmul(
                            avp[:, qt, hh, :],
                            lhsT=attn_list[ci][:, h, qt * P:(qt + 1) * P],
                            rhs=V_sb[:, c0 + ci, h * HD:(h + 1) * HD],
                            start=(ci == 0),
                            stop=(ci == KC_F - 1),
                        )
            sl = slice(4 * g, 4 * g + 4)
            if first:
                nc.vector.tensor_copy(ctxq32[:, :, sl, :], avp[:])
            elif last:
                with nc.allow_low_precision(reason="final ctx to bf16"):
                    nc.vector.tensor_add(ctxq_bf[:, :, sl, :],
                                         ctxq32[:, :, sl, :], avp[:])
            else:
                nc.vector.tensor_add(ctxq32[:, :, sl, :], ctxq32[:, :, sl, :],
                                     avp[:])

        if not AV_FLIP:
            prev = None  # (c0, attn_list)
            for ch in range(NKT // KC):
                c0 = ch * KC
                cur = []
                for ci in range(KC):
                    cur.append(softmax_kt(c0 + ci))
                    if prev is not None:
                        for u in (2 * ci, 2 * ci + 1):
                            av_group(u, prev[0], prev[1], prev[0] == 0, False)
                prev = (c0, cur)
            for u in range(4):
                av_group(u, prev[0], prev[1], False, True)
        else:
            prev = None
            for ch in range(NKT // KC_F):
                c0 = ch * KC_F
                cur = []
                for ci in range(KC_F):
                    cur.append(softmax_kt(c0 + ci))
                    if prev is not None:
                        av_flip_group(ci, prev[0], prev[1], prev[0] == 0,
                                      False)
                prev = (c0, cur)
            for g in range(4):
                av_flip_group(g, prev[0], prev[1], False, True)
            # transpose ctx [q, d] -> ctx_sb [d, q] via PE (d-block = 2 heads)
            for s in range(S):
                for qt in range(4):
                    tp = tp_psum.tile([P, P], MM_DT, tag="tp")
                    nc.tensor.transpose(tp[:], ctxq_bf[:, qt, 2 * s:2 * s + 2, :],
                                        ident_t[:])
                    if qt % 2 == 0:
                        nc.scalar.copy(ctx_sb[:, s, qt * P:(qt + 1) * P], tp[:])
                    else:
                        nc.vector.tensor_copy(
                            ctx_sb[:, s, qt * P:(qt + 1) * P], tp[:])

        # ---------------- phase 3: output projection ----------------
        for j4 in range(2):
            woh = wo_pool.tile([P, S, 512], MM_DT, tag="wo")
            nc.sync.dma_start(woh[:], wo_ap[:, j4])
            for j2 in range(2):
                po = e_psum.tile([P, 2, LQ], F32, tag="e")
                for jj in range(2):
                    j = j4 * 4 + j2 * 2 + jj
                    jl = j2 * 2 + jj
                    for s in range(S):
                        nc.tensor.matmul(
                            po[:, jj, :],
                            lhsT=woh[:, s, jl * P:(jl + 1) * P],
                            rhs=ctx_sb[:, s, :],
                            start=(s == 0),
                            stop=(s == S - 1 and not has_bias["bo"]),
                        )
                    if has_bias["bo"]:
                        bias_mm(po[:, jj, :], "bo", j * P, LQ, True)
                osb = osb_pool.tile([P, 2, LQ], F32)
                nc.scalar.copy(osb[:], po[:])
                j0 = j4 * 4 + j2 * 2
                nc.sync.dma_start(outT_ap[:, j0:j0 + 2, :], osb[:])

    nc.compile()
    return nc


_cache = {}


def _get_program(has_bias):
    key = (BENCH_LOOP, AV_FLIP, tuple(sorted(has_bias.items())))
    if key not in _cache:
        _cache[key] = _build(has_bias)
    return _cache[key]


import ml_dtypes

NP_BF16 = ml_dtypes.bfloat16


def _part_major(x):
    n = x.shape[1]
    return np.ascontiguousarray(
        x.reshape(S, P, n).transpose(1, 0, 2).reshape(P, S * n)
        .astype(NP_BF16))


def _chunked(x, width=512):
    """[D, N] -> [P, N//width, S, width] per-chunk contiguous layout."""
    n = x.shape[1]
    nch = n // width
    y = x.reshape(S, P, nch, width).transpose(1, 2, 0, 3)
    return np.ascontiguousarray(y.reshape(P, nch * S * width).astype(NP_BF16))


def prepare_inputs(query, key, value, Wq_w, Wq_b, Wk_w, Wk_b, Wv_w, Wv_b,
                   Wo_w, Wo_b):
    query = np.asarray(query, dtype=np.float32)
    key = np.asarray(key, dtype=np.float32)
    value = np.asarray(value, dtype=np.float32)
    w = {
        "wq": _chunked(np.ascontiguousarray(np.asarray(Wq_w, np.float32).T)),
        "wk": _chunked(np.ascontiguousarray(np.asarray(Wk_w, np.float32).T)),
        "wv": _chunked(np.ascontiguousarray(np.asarray(Wv_w, np.float32).T)),
        "wo": _chunked(np.ascontiguousarray(np.asarray(Wo_w, np.float32).T)),
    }
    biases = {"bq": np.asarray(Wq_b, np.float32), "bk": np.asarray(Wk_b, np.float32),
              "bv": np.asarray(Wv_b, np.float32), "bo": np.asarray(Wo_b, np.float32)}
    has_bias = {nm: bool(np.any(b)) for nm, b in biases.items()}

    kT = [_chunked(np.ascontiguousarray(key[b].T)) for b in range(B)]
    vT = [_chunked(np.ascontiguousarray(value[b].T)) for b in range(B)]
    ident = np.eye(P, dtype=NP_BF16)

    in_maps = []
    for c in range(N_CORES):
        b, qc = c // (N_CORES // B), c % (N_CORES // B)
        qslice = query[b, qc * LQ:(qc + 1) * LQ, :]
        m = {
            "qT": _part_major(np.ascontiguousarray(qslice.T)),
            "kT": kT[b],
            "vT": vT[b],
            **w,
        }
        if AV_FLIP:
            m["ident"] = ident
        for nm, hb in has_bias.items():
            if hb:
                m[nm] = biases[nm].reshape(1, D)
        in_maps.append(m)
    return in_maps, has_bias


def gather_output(results):
    out = np.empty((B, L, D), dtype=np.float32)
    for c in range(N_CORES):
        b, qc = c // (N_CORES // B), c % (N_CORES // B)
        oT = results[c]["outT"].reshape(P, S, LQ).transpose(1, 0, 2).reshape(D, LQ)
        out[b, qc * LQ:(qc + 1) * LQ, :] = oT.T
    return out


def kernel(**inputs) -> np.ndarray:
    in_maps, has_bias = prepare_inputs(**inputs)
    nc = _get_program(has_bias)
    res = run_bass_kernel_spmd(nc, in_maps, list(range(N_CORES)))
    return gather_output(res.results)



# revision 22
# speedup vs baseline: 1.2456x; 1.2456x over previous
"""CategoryAttention (softmax over heads axis) on 8 Trainium2 cores.

Sharding: B*L = 2*2048 = 4096 query rows split 8 ways (512 rows/core).
Core c handles batch b=c//4, query rows [(c%4)*512, (c%4+1)*512).
The softmax is over the 16 heads, which is fully local to each (q,k)
position, so no cross-core communication is needed. Each core
recomputes K/V projections for its batch (4x redundant).

Per-core pipeline (layouts transposed so the model dim rides the SBUF
partition axis; all big matmuls in float32r at full PE rate):
  phase 1: Q^T = Wq^T.T @ q^T;  K^T = Wk^T.T @ k^T;  V = v^T.T @ Wv^T
  phase 2: per k-tile (q = all 512 rows at once):
    e_h^T[k,q] = Kh^T.T @ Qh^T   (16 heads; 2-head psum tiles x3 so
                                  the PE/ACT exp pipeline overlaps)
    p_h = exp(e_h/8)             (ACT, psum->sbuf, bf16)
    den = sum_h p_h              (DVE bf16 tree, f32 final add)
    attn = p * (1/den)           (DVE approx-recip + bcast mult at 2x,
                                  a slice of heads on GPSIMD)
    ctx_h^T += Vh.T @ attn_h^T   (PSUM accum over KC=2 k-tiles,
                                  1 full bank per head-pair, DVE drain)
  phase 3: out^T = Wo^T.T @ ctx^T + bias
"""

import os

import numpy as np
from contextlib import ExitStack

import concourse.bass as bass
import concourse.tile as tile
from concourse import bacc, mybir
from concourse.bass_utils import run_bass_kernel_spmd

F32 = mybir.dt.float32
F32R = mybir.dt.float32r
BF16 = mybir.dt.bfloat16

N_CORES = 8
P = 128
D = 1024          # d_model
S = D // P        # 8 subtiles of the contraction dim
H = 16            # heads
HD = 64           # head dim
B = 2
L = 2048
LQ = L * B // N_CORES   # 512 query rows per core
LK = L                  # key rows per core (full batch slice)
KTS = 128               # k tile
NKT = LK // KTS         # 16
KC = 2                  # k-tiles per AV psum accumulation chunk
KC_F = 4                # k-tiles per chunk in the flipped-AV path
GP_HEADS = int(os.environ.get("GP_HEADS", "0"))  # heads normalized on GPSIMD
SCALE = 1.0 / np.sqrt(HD)

ATT_DT = BF16           # exp/attn planes and V dtype
MM_DT = BF16            # all matmul operands (f32 psum accumulate)

BENCH_LOOP = int(os.environ.get("BENCH_LOOP", "1"))
AV_FLIP = int(os.environ.get("AV_FLIP", "0"))  # attn-as-weights AV path
V3 = int(os.environ.get("V3", "1"))  # interleaved proj/attention schedule


def _build_v3(has_bias):
    """Interleaved schedule: K/V projection chunks woven between attention
    k-tiles so ACT/DVE softmax work overlaps PE projection work.

    chunk = 4 k-tiles (512 k rows). Iteration it: project chunk `it` (PE,
    drains on ACT), softmax k-tiles of chunk it-1 (PE energy -> ACT exp ->
    DVE tree/recip/normalize), then AV for chunk it-1 (PE, psum accumulated
    over the 4 k-tiles, single DVE drain-add per 4-head group)."""
    nc = bacc.Bacc("TRN2", target_bir_lowering=False, debug=False, num_devices=1)

    def din(name, shape, dt=F32):
        return nc.dram_tensor(name, shape, dt, kind="ExternalInput").ap()

    qT_d = din("qT", (P, S * LQ), MM_DT)
    kT_d = din("kT", (P, S * LK), MM_DT)
    vT_d = din("vT", (P, S * LK), MM_DT)
    wq_d = din("wq", (P, S * D), MM_DT)
    wk_d = din("wk", (P, S * D), MM_DT)
    wv_d = din("wv", (P, S * D), MM_DT)
    wo_d = din("wo", (P, S * D), MM_DT)
    bias_d = {}
    for nm in ("bq", "bk", "bv", "bo"):
        if has_bias[nm]:
            bias_d[nm] = din(nm, (1, D))
    outT_d = nc.dram_tensor("outT", (P, S * LQ), F32, kind="ExternalOutput").ap()

    qT_ap = qT_d.rearrange("p (s q) -> p s q", s=S)
    kT_ap = kT_d.rearrange("p (c s k) -> p c s k", c=4, s=S)
    vT_ap = vT_d.rearrange("p (c s k) -> p c s k", c=4, s=S)
    wq_ap = wq_d.rearrange("p (h s o) -> p h s o", h=2, s=S)
    wk_ap = wk_d.rearrange("p (h s o) -> p h s o", h=2, s=S)
    wv_ap = wv_d.rearrange("p (h s o) -> p h s o", h=2, s=S)
    wo_ap = wo_d.rearrange("p (h s o) -> p h s o", h=2, s=S)
    outT_ap = outT_d.rearrange("p (j q) -> p j q", j=S)

    with tile.TileContext(nc) as tc, ExitStack() as ctx:
        if BENCH_LOOP > 1:
            ctx.enter_context(tc.For_i(0, BENCH_LOOP, 1))

        const_pool = ctx.enter_context(tc.tile_pool(name="const", bufs=1))
        qt_pool = ctx.enter_context(tc.tile_pool(name="QT", bufs=1))
        kt_pool = ctx.enter_context(tc.tile_pool(name="KT", bufs=1))
        v_pool = ctx.enter_context(tc.tile_pool(name="V", bufs=1))
        stream = ctx.enter_context(tc.tile_pool(name="stream", bufs=4))
        attn_pool = ctx.enter_context(tc.tile_pool(name="attn", bufs=4))
        tree_pool = ctx.enter_context(tc.tile_pool(name="tree", bufs=1))
        den_pool = ctx.enter_context(tc.tile_pool(name="den", bufs=1))
        r_pool = ctx.enter_context(tc.tile_pool(name="r", bufs=1))
        rb_pool = ctx.enter_context(tc.tile_pool(name="rb", bufs=2))
        ctx_pool = ctx.enter_context(tc.tile_pool(name="ctx", bufs=1))
        osb_pool = ctx.enter_context(tc.tile_pool(name="osb", bufs=2))
        mm_psum = ctx.enter_context(tc.tile_pool(name="mmpsum", bufs=2,
                                                 space="PSUM"))
        av_psum = ctx.enter_context(tc.tile_pool(name="avpsum", bufs=2,
                                                 space="PSUM"))

        any_bias = any(has_bias.values())
        ones_t = None
        if any_bias:
            ones_t = const_pool.tile([1, 512], F32, tag="ones")
            nc.vector.memset(ones_t[:], 1.0)
        bias_t = {}
        for nm, d_ap in bias_d.items():
            t = const_pool.tile([1, D], F32, tag=f"bias_{nm}")
            nc.sync.dma_start(t[:], d_ap)
            bias_t[nm] = t

        QT_sb = qt_pool.tile([P, S, LQ], MM_DT)
        KT_sb = kt_pool.tile([P, S, LK], MM_DT)
        V_sb = v_pool.tile([P, NKT, D], ATT_DT)
        ctx_sb = ctx_pool.tile([P, S, LQ], MM_DT)

        def bias_mm(ps_t, bias_name, o0, n_sz, o_on_partitions):
            if o_on_partitions:
                nc.tensor.matmul(ps_t, lhsT=bias_t[bias_name][0:1, o0:o0 + P],
                                 rhs=ones_t[0:1, :n_sz], start=False, stop=True)
            else:
                nc.tensor.matmul(ps_t, lhsT=ones_t[0:1, 0:P],
                                 rhs=bias_t[bias_name][0:1, o0:o0 + n_sz],
                                 start=False, stop=True)

        def stream_tile(src_ap, tag="w"):
            t = stream.tile([P, S, 512], MM_DT, tag=tag)
            nc.sync.dma_start(t[:], src_ap)
            return t

        def proj2(w_t, wj0, in_t, out_view, bias_name, ob0):
            """Two 128-row output tiles through one mm psum tile (ACT drain).

            out^T[o, n] = sum_s W^T[s-block, o] @ in^T[s-block, n]."""
            ps = mm_psum.tile([P, 2, 512], F32, tag="mm")
            for jl in range(2):
                for s in range(S):
                    nc.tensor.matmul(
                        ps[:, jl, :],
                        lhsT=w_t[:, s, (wj0 + jl) * P:(wj0 + jl + 1) * P],
                        rhs=in_t[:, s, :],
                        start=(s == 0),
                        stop=(s == S - 1 and bias_name is None),
                    )
                if bias_name is not None:
                    bias_mm(ps[:, jl, :], bias_name, (ob0 + jl) * P, 512, True)
            nc.scalar.copy(out_view, ps[:])

        def projV(vin_t, wv_t2, kt, bias_name):
            """V row-block kt: out[k-part, 1024] via 2 d-halves."""
            ps = mm_psum.tile([P, 2, 512], F32, tag="mm")
            for t in range(2):
                for s in range(S):
                    nc.tensor.matmul(
                        ps[:, t, :],
                        lhsT=vin_t[:, s, (kt % 4) * P:(kt % 4 + 1) * P],
                        rhs=wv_t2[t][:, s, :],
                        start=(s == 0),
                        stop=(s == S - 1 and bias_name is None),
                    )
                if bias_name is not None:
                    bias_mm(ps[:, t, :], bias_name, t * 512, 512, False)
            nc.scalar.copy(V_sb[:, kt, :],
                           ps[:].rearrange("p a b -> p (a b)"))

        def softmax_kt(kt):
            """Energy (16 heads) -> exp -> den -> normalized attn tile."""
            attn_t = attn_pool.tile([P, H, LQ], ATT_DT, tag="attn")
            for g in range(8):
                eps = mm_psum.tile([P, 2, LQ], F32, tag="mm")
                for hh in range(2):
                    h = g * 2 + hh
                    j2, p0 = h // 2, HD * (h % 2)
                    nc.tensor.matmul(
                        eps[:, hh, :],
                        lhsT=KT_sb[p0:p0 + HD, j2, kt * KTS:(kt + 1) * KTS],
                        rhs=QT_sb[p0:p0 + HD, j2, :],
                        start=True,
                        stop=True,
                    )
                nc.scalar.activation(attn_t[:, g * 2:(g + 1) * 2, :], eps[:],
                                     mybir.ActivationFunctionType.Exp,
                                     scale=float(SCALE))
            # den = sum over heads (bf16 tree at DVE 2x)
            t1 = tree_pool.tile([P, 4, LQ], ATT_DT)
            with nc.allow_low_precision(reason="bf16 head-sum tree"):
                nc.vector.tensor_add(t1[:], attn_t[:, 0:4, :], attn_t[:, 4:8, :])
                nc.vector.tensor_add(t1[:], t1[:], attn_t[:, 8:12, :])
                nc.vector.tensor_add(t1[:], t1[:], attn_t[:, 12:16, :])
                nc.vector.tensor_add(t1[:, 0:2, :], t1[:, 0:2, :], t1[:, 2:4, :])
            den = den_pool.tile([P, LQ], F32)
            nc.vector.tensor_add(den[:], t1[:, 0, :], t1[:, 1, :])
            r32 = r_pool.tile([P, LQ], F32)
            nc.vector.reciprocal_approx_fast(r32[:], den[:])
            rb = rb_pool.tile([P, LQ], ATT_DT)
            nc.vector.tensor_copy(rb[:], r32[:])
            nd = H - GP_HEADS
            nc.vector.tensor_mul(
                attn_t[:, 0:nd, :], attn_t[:, 0:nd, :],
                rb[:, None, :].to_broadcast((P, nd, LQ)))
            if GP_HEADS:
                nc.gpsimd.tensor_mul(
                    attn_t[:, nd:H, :], attn_t[:, nd:H, :],
                    rb[:, None, :].to_broadcast((P, GP_HEADS, LQ)))
            return attn_t

        def av_group(u, c0, attn_list, first, last):
            """Heads 4u..4u+3, psum-accumulated over the chunk's 4 k-tiles."""
            avp = av_psum.tile([P, 2, LQ], F32, tag="av")
            for ci in range(4):
                kt = c0 + ci
                for hh in range(4):
                    h = 4 * u + hh
                    i, p0 = hh // 2, HD * (hh % 2)
                    nc.tensor.matmul(
                        avp[p0:p0 + HD, i, :],
                        lhsT=V_sb[:, kt, h * HD:(h + 1) * HD],
                        rhs=attn_list[ci][:, h, :],
                        start=(ci == 0),
                        stop=(ci == 3),
                    )
            sl = slice(2 * u, 2 * u + 2)
            if first:
                nc.vector.tensor_copy(ctx_sb[:, sl, :], avp[:])
            else:
                with nc.allow_low_precision(reason="bf16 ctx accumulate"):
                    nc.vector.tensor_add(ctx_sb[:, sl, :], ctx_sb[:, sl, :],
                                         avp[:])

        # ---- Q projection ----
        qin = stream_tile(qT_ap)
        wq_h = [stream_tile(wq_ap[:, wh]) for wh in range(2)]
        bq = "bq" if has_bias["bq"] else None
        for ci in range(4):
            proj2(wq_h[ci // 2], (ci % 2) * 2, qin,
                  QT_sb[:, 2 * ci:2 * ci + 2, :], bq, 2 * ci)

        # ---- interleaved chunk loop ----
        bk = "bk" if has_bias["bk"] else None
        bv = "bv" if has_bias["bv"] else None
        for it in range(5):
            if it < 4:
                kin = stream_tile(kT_ap[:, it])
                wk0 = stream_tile(wk_ap[:, 0])
                wk1 = stream_tile(wk_ap[:, 1])
            cur = []
            for ci in range(4):
                if it < 4:
                    wk_t = wk0 if ci < 2 else wk1
                    proj2(wk_t, (ci % 2) * 2, kin,
                          KT_sb[:, 2 * ci:2 * ci + 2, it * 512:(it + 1) * 512],
                          bk, 2 * ci)
                if it >= 1:
                    cur.append(softmax_kt(4 * (it - 1) + ci))
            if it < 4:
                vin = stream_tile(vT_ap[:, it])
                wv_t2 = [stream_tile(wv_ap[:, 0]), stream_tile(wv_ap[:, 1])]
                for ci in range(4):
                    projV(vin, wv_t2, 4 * it + ci, bv)
            if it >= 1:
                for u in range(4):
                    av_group(u, 4 * (it - 1), cur, it == 1, it == 4)

        # ---- output projection ----
        bo = "bo" if has_bias["bo"] else None
        wo_h = [stream_tile(wo_ap[:, 0]), stream_tile(wo_ap[:, 1])]
        for g in range(4):
            po = mm_psum.tile([P, 2, LQ], F32, tag="mm")
            for jl in range(2):
                j = 2 * g + jl
                for s in range(S):
                    nc.tensor.matmul(
                        po[:, jl, :],
                        lhsT=wo_h[j // 4][:, s, (j % 4) * P:(j % 4 + 1) * P],
                        rhs=ctx_sb[:, s, :],
                        start=(s == 0),
                        stop=(s == S - 1 and bo is None),
                    )
                if bo is not None:
                    bias_mm(po[:, jl, :], "bo", j * P, LQ, True)
            osb = osb_pool.tile([P, 2, LQ], F32, tag="osb")
            nc.scalar.copy(osb[:], po[:])
            nc.sync.dma_start(outT_ap[:, 2 * g:2 * g + 2, :], osb[:])

    nc.compile()
    return nc


def _build(has_bias):
    if V3:
        return _build_v3(has_bias)
    nc = bacc.Bacc("TRN2", target_bir_lowering=False, debug=False, num_devices=1)

    def din(name, shape, dt=F32):
        return nc.dram_tensor(name, shape, dt, kind="ExternalInput").ap()

    qT_d = din("qT", (P, S * LQ), MM_DT)
    kT_d = din("kT", (P, S * LK), MM_DT)
    vT_d = din("vT", (P, S * LK), MM_DT)
    wq_d = din("wq", (P, S * D), MM_DT)
    wk_d = din("wk", (P, S * D), MM_DT)
    wv_d = din("wv", (P, S * D), MM_DT)
    wo_d = din("wo", (P, S * D), MM_DT)
    bias_d = {}
    for nm in ("bq", "bk", "bv", "bo"):
        if has_bias[nm]:
            bias_d[nm] = din(nm, (1, D))
    ident_d = din("ident", (P, P), MM_DT) if AV_FLIP else None
    outT_d = nc.dram_tensor("outT", (P, S * LQ), F32, kind="ExternalOutput").ap()

    qT_ap = qT_d.rearrange("p (s q) -> p s q", s=S)
    kT_ap = kT_d.rearrange("p (c s k) -> p c s k", c=4, s=S)
    vT_ap = vT_d.rearrange("p (c s k) -> p c s k", c=4, s=S)
    wq_ap = wq_d.rearrange("p (h s o) -> p h s o", h=2, s=S)
    wk_ap = wk_d.rearrange("p (h s o) -> p h s o", h=2, s=S)
    wv_ap = wv_d.rearrange("p (h s o) -> p h s o", h=2, s=S)
    wo_ap = wo_d.rearrange("p (h s o) -> p h s o", h=2, s=S)
    outT_ap = outT_d.rearrange("p (j q) -> p j q", j=S)

    with tile.TileContext(nc) as tc, ExitStack() as ctx:
        if BENCH_LOOP > 1:
            ctx.enter_context(tc.For_i(0, BENCH_LOOP, 1))

        const_pool = ctx.enter_context(tc.tile_pool(name="const", bufs=1))
        qt_pool = ctx.enter_context(tc.tile_pool(name="QT", bufs=1))
        kt_pool = ctx.enter_context(tc.tile_pool(name="KT", bufs=1))
        v_pool = ctx.enter_context(tc.tile_pool(name="V", bufs=1))

        any_bias = any(has_bias.values())
        ones_t = None
        if any_bias:
            ones_t = const_pool.tile([1, 512], F32, tag="ones")
            nc.vector.memset(ones_t[:], 1.0)
        bias_t = {}
        for nm, d_ap in bias_d.items():
            t = const_pool.tile([1, D], F32, tag=f"bias_{nm}")
            nc.sync.dma_start(t[:], d_ap)
            bias_t[nm] = t

        QT_sb = qt_pool.tile([P, S, LQ], MM_DT)
        KT_sb = kt_pool.tile([P, S, LK], MM_DT)
        V_sb = v_pool.tile([P, NKT, D], ATT_DT)

        def bias_mm(ps_t, bias_name, o0, n_sz, o_on_partitions):
            if o_on_partitions:
                nc.tensor.matmul(ps_t, lhsT=bias_t[bias_name][0:1, o0:o0 + P],
                                 rhs=ones_t[0:1, :n_sz], start=False, stop=True)
            else:
                nc.tensor.matmul(ps_t, lhsT=ones_t[0:1, 0:P],
                                 rhs=bias_t[bias_name][0:1, o0:o0 + n_sz],
                                 start=False, stop=True)

        # ---------------- phase 1: Q, K, V projections ----------------
        with tc.tile_pool(name="stream", bufs=5) as spool, \
             tc.tile_pool(name="ppsum", bufs=2, space="PSUM") as ppsum:

            def stream_tile(src_ap):
                t = spool.tile([P, S, 512], MM_DT, tag="w")
                nc.sync.dma_start(t[:], src_ap)
                return t

            def proj_cols(w_halves, in_t, out_view, bias_name, n_sz):
                # out^T[o,n] = sum_i W^T[i,o]*in^T[i,n]; 4 o-tiles per psum
                # tile (one bank per matmul target), one ACT copy out.
                for j4 in range(2):
                    ps = ppsum.tile([P, 4, 512], F32, tag="pp")
                    for jl in range(4):
                        j = j4 * 4 + jl
                        w_t = w_halves[j // 4]
                        for s in range(S):
                            nc.tensor.matmul(
                                ps[:, jl, :n_sz],
                                lhsT=w_t[:, s, jl * P:(jl + 1) * P],
                                rhs=in_t[:, s, :n_sz],
                                start=(s == 0),
                                stop=(s == S - 1 and bias_name is None),
                            )
                        if bias_name is not None:
                            bias_mm(ps[:, jl, :n_sz], bias_name, j * P, n_sz,
                                    True)
                    nc.scalar.copy(out_view[:, j4 * 4:(j4 + 1) * 4, :n_sz],
                                   ps[:, :, :n_sz])

            qin = stream_tile(qT_ap)
            wq_h = [stream_tile(wq_ap[:, wh]) for wh in range(2)]
            proj_cols(wq_h, qin, QT_sb, "bq" if has_bias["bq"] else None, LQ)

            wk_h = [stream_tile(wk_ap[:, wh]) for wh in range(2)]
            for kn in range(4):
                kin = stream_tile(kT_ap[:, kn])
                proj_cols(wk_h, kin, KT_sb[:, :, kn * 512:(kn + 1) * 512],
                          "bk" if has_bias["bk"] else None, 512)

            wv_h = [stream_tile(wv_ap[:, wh]) for wh in range(2)]
            for kn in range(4):
                vin = stream_tile(vT_ap[:, kn])
                for kt4 in range(4):
                    kt = kn * 4 + kt4
                    ps = ppsum.tile([P, 4, 512], F32, tag="pp")
                    for t in range(2):  # o halves; 2 targets used of 4
                        for s in range(S):
                            nc.tensor.matmul(
                                ps[:, t, :],
                                lhsT=vin[:, s, kt4 * P:(kt4 + 1) * P],
                                rhs=wv_h[t][:, s, :],
                                start=(s == 0),
                                stop=(s == S - 1 and not has_bias["bv"]),
                            )
                        if has_bias["bv"]:
                            bias_mm(ps[:, t, :], "bv", t * 512, 512, False)
                    nc.scalar.copy(V_sb[:, kt, :],
                                   ps[:, 0:2, :].rearrange("p a b -> p (a b)"))

        # ---------------- phase 2: attention (q = 512) ----------------
        attn_pool = ctx.enter_context(
            tc.tile_pool(name="attn", bufs=5 if AV_FLIP else 3))
        tree_pool = ctx.enter_context(tc.tile_pool(name="tree", bufs=1))
        den_pool = ctx.enter_context(tc.tile_pool(name="den", bufs=1))
        r_pool = ctx.enter_context(tc.tile_pool(name="r", bufs=1))
        rb_pool = ctx.enter_context(tc.tile_pool(name="rb", bufs=1))
        ctx_pool = ctx.enter_context(tc.tile_pool(name="ctx", bufs=1))
        osb_pool = ctx.enter_context(tc.tile_pool(name="osb", bufs=1))
        wo_pool = ctx.enter_context(tc.tile_pool(name="wo", bufs=1))
        e_psum = ctx.enter_context(
            tc.tile_pool(name="epsum", bufs=2 if AV_FLIP else 3, space="PSUM"))
        av_psum = ctx.enter_context(
            tc.tile_pool(name="avpsum", bufs=1, space="PSUM"))
        if AV_FLIP:
            tp_psum = ctx.enter_context(
                tc.tile_pool(name="tppsum", bufs=2, space="PSUM"))

        # f32 ctx accumulator + bf16 copy for the O-proj rhs
        ctx_sb = ctx_pool.tile([P, S, LQ], MM_DT, tag="ctxbf")
        if AV_FLIP:
            # (qt, head, hd) layout: d-blocks of 2 heads contiguous per qt
            ctxq32 = ctx_pool.tile([P, 4, H, HD], F32, tag="cq32")
            ctxq_bf = ctx_pool.tile([P, 4, H, HD], MM_DT, tag="cqbf")
            ident_t = const_pool.tile([P, P], MM_DT, tag="ident")
            nc.sync.dma_start(ident_t[:], ident_d)
        else:
            ctx32_sb = ctx_pool.tile([P, S, LQ], F32)

        def softmax_kt(kt):
            """Energy (16 heads) -> exp -> den -> normalized attn tile."""
            attn_t = attn_pool.tile([P, H, LQ], ATT_DT, tag="attn")
            for g in range(8):  # 2 heads per psum tile, one bank per head
                eps = e_psum.tile([P, 2, LQ], F32, tag="e")
                for hh in range(2):
                    h = g * 2 + hh
                    j2, p0 = h // 2, HD * (h % 2)
                    nc.tensor.matmul(
                        eps[:, hh, :],
                        lhsT=KT_sb[p0:p0 + HD, j2, kt * KTS:(kt + 1) * KTS],
                        rhs=QT_sb[p0:p0 + HD, j2, :],
                        start=True,
                        stop=True,
                    )
                nc.scalar.activation(attn_t[:, g * 2:(g + 1) * 2, :], eps[:],
                                     mybir.ActivationFunctionType.Exp,
                                     scale=float(SCALE))
            # den = sum over heads (bf16 tree at DVE 2x; tail on GPSIMD)
            t1 = tree_pool.tile([P, 4, LQ], ATT_DT)
            with nc.allow_low_precision(reason="bf16 head-sum tree"):
                nc.vector.tensor_add(t1[:], attn_t[:, 0:4, :], attn_t[:, 4:8, :])
                nc.vector.tensor_add(t1[:], t1[:], attn_t[:, 8:12, :])
                nc.vector.tensor_add(t1[:], t1[:], attn_t[:, 12:16, :])
                nc.vector.tensor_add(t1[:, 0:2, :], t1[:, 0:2, :], t1[:, 2:4, :])
            den = den_pool.tile([P, LQ], F32)
            nc.vector.tensor_add(den[:], t1[:, 0, :], t1[:, 1, :])
            r32 = r_pool.tile([P, LQ], F32)
            nc.vector.reciprocal_approx_fast(r32[:], den[:])
            rb = rb_pool.tile([P, LQ], ATT_DT)
            nc.gpsimd.tensor_copy(rb[:], r32[:])
            nd = H - GP_HEADS
            nc.vector.tensor_mul(
                attn_t[:, 0:nd, :], attn_t[:, 0:nd, :],
                rb[:, None, :].to_broadcast((P, nd, LQ)))
            if GP_HEADS:
                nc.gpsimd.tensor_mul(
                    attn_t[:, nd:H, :], attn_t[:, nd:H, :],
                    rb[:, None, :].to_broadcast((P, GP_HEADS, LQ)))
            return attn_t

        def av_group(u, c0, attn_list, first, last):
            """One avp tile: heads 4u..4u+3, full q, over KC k-tiles."""
            avp = av_psum.tile([P, 2, LQ], F32, tag="av")
            for ci in range(KC):
                kt = c0 + ci
                for hh in range(4):
                    h = 4 * u + hh
                    i, p0 = hh // 2, HD * (hh % 2)
                    nc.tensor.matmul(
                        avp[p0:p0 + HD, i, :],
                        lhsT=V_sb[:, kt, h * HD:(h + 1) * HD],
                        rhs=attn_list[ci][:, h, :],
                        start=(ci == 0),
                        stop=(ci == KC - 1),
                    )
            sl = slice(2 * u, 2 * u + 2)
            if first:
                nc.vector.tensor_copy(ctx32_sb[:, sl, :], avp[:, :, :])
            elif last:
                with nc.allow_low_precision(reason="final ctx to bf16"):
                    nc.vector.tensor_add(ctx_sb[:, sl, :], ctx32_sb[:, sl, :],
                                         avp[:, :, :])
            else:
                nc.vector.tensor_add(ctx32_sb[:, sl, :], ctx32_sb[:, sl, :],
                                     avp[:, :, :])

        def av_flip_group(g, c0, attn_list, first, last):
            """Flipped AV: attn tiles are PE weights; heads 4g..4g+3.

            out psum [128 q, hh, qt, 64hd] accumulated over the chunk's
            KC_F k-tiles; ctx kept in [q-part, head, qt, hd] layout."""
            avp = av_psum.tile([P, 4, 4, HD], F32, tag="av")  # (qt, hh, hd)
            for hh in range(4):
                h = 4 * g + hh
                for qt in range(4):
                    for ci in range(KC_F):
                        nc.tensor.matmul(
                            avp[:, qt, hh, :],
                            lhsT=attn_list[ci][:, h, qt * P:(qt + 1) * P],
                            rhs=V_sb[:, c0 + ci, h * HD:(h + 1) * HD],
                            start=(ci == 0),
                            stop=(ci == KC_F - 1),
                        )
            sl = slice(4 * g, 4 * g + 4)
            if first:
                nc.vector.tensor_copy(ctxq32[:, :, sl, :], avp[:])
            elif last:
                with nc.allow_low_precision(reason="final ctx to bf16"):
                    nc.vector.tensor_add(ctxq_bf[:, :, sl, :],
                                         ctxq32[:, :, sl, :], avp[:])
            else:
                nc.vector.tensor_add(ctxq32[:, :, sl, :], ctxq32[:, :, sl, :],
                                     avp[:])

        if not AV_FLIP:
            prev = None  # (c0, attn_list)
            for ch in range(NKT // KC):
                c0 = ch * KC
                cur = []
                for ci in range(KC):
                    cur.append(softmax_kt(c0 + ci))
                    if prev is not None:
                        for u in (2 * ci, 2 * ci + 1):
                            av_group(u, prev[0], prev[1], prev[0] == 0, False)
                prev = (c0, cur)
            for u in range(4):
                av_group(u, prev[0], prev[1], False, True)
        else:
            prev = None
            for ch in range(NKT // KC_F):
                c0 = ch * KC_F
                cur = []
                for ci in range(KC_F):
                    cur.append(softmax_kt(c0 + ci))
                    if prev is not None:
                        av_flip_group(ci, prev[0], prev[1], prev[0] == 0,
                                      False)
                prev = (c0, cur)
            for g in range(4):
                av_flip_group(g, prev[0], prev[1], False, True)
            # transpose ctx [q, d] -> ctx_sb [d, q] via PE (d-block = 2 heads)
            for s in range(S):
                for qt in range(4):
                    tp = tp_psum.tile([P, P], MM_DT, tag="tp")
                    nc.tensor.transpose(tp[:], ctxq_bf[:, qt, 2 * s:2 * s + 2, :],
                                        ident_t[:])
                    if qt % 2 == 0:
                        nc.scalar.copy(ctx_sb[:, s, qt * P:(qt + 1) * P], tp[:])
                    else:
                        nc.vector.tensor_copy(
                            ctx_sb[:, s, qt * P:(qt + 1) * P], tp[:])

        # ---------------- phase 3: output projection ----------------
        for j4 in range(2):
            woh = wo_pool.tile([P, S, 512], MM_DT, tag="wo")
            nc.sync.dma_start(woh[:], wo_ap[:, j4])
            for j2 in range(2):
                po = e_psum.tile([P, 2, LQ], F32, tag="e")
                for jj in range(2):
                    j = j4 * 4 + j2 * 2 + jj
                    jl = j2 * 2 + jj
                    for s in range(S):
                        nc.tensor.matmul(
                            po[:, jj, :],
                            lhsT=woh[:, s, jl * P:(jl + 1) * P],
                            rhs=ctx_sb[:, s, :],
                            start=(s == 0),
                            stop=(s == S - 1 and not has_bias["bo"]),
                        )
                    if has_bias["bo"]:
                        bias_mm(po[:, jj, :], "bo", j * P, LQ, True)
                osb = osb_pool.tile([P, 2, LQ], F32)
                nc.scalar.copy(osb[:], po[:])
                j0 = j4 * 4 + j2 * 2
                nc.sync.dma_start(outT_ap[:, j0:j0 + 2, :], osb[:])

    nc.compile()
    return nc


_cache = {}


def _get_program(has_bias):
    key = (BENCH_LOOP, AV_FLIP, tuple(sorted(has_bias.items())))
    if key not in _cache:
        _cache[key] = _build(has_bias)
    return _cache[key]


import ml_dtypes

NP_BF16 = ml_dtypes.bfloat16


def _part_major(x):
    n = x.shape[1]
    return np.ascontiguousarray(
        x.reshape(S, P, n).transpose(1, 0, 2).reshape(P, S * n)
        .astype(NP_BF16))


def _chunked(x, width=512):
    """[D, N] -> [P, N//width, S, width] per-chunk contiguous layout."""
    n = x.shape[1]
    nch = n // width
    y = x.reshape(S, P, nch, width).transpose(1, 2, 0, 3)
    return np.ascontiguousarray(y.reshape(P, nch * S * width).astype(NP_BF16))


def prepare_inputs(query, key, value, Wq_w, Wq_b, Wk_w, Wk_b, Wv_w, Wv_b,
                   Wo_w, Wo_b):
    query = np.asarray(query, dtype=np.float32)
    key = np.asarray(key, dtype=np.float32)
    value = np.asarray(value, dtype=np.float32)
    w = {
        "wq": _chunked(np.ascontiguousarray(np.asarray(Wq_w, np.float32).T)),
        "wk": _chunked(np.ascontiguousarray(np.asarray(Wk_w, np.float32).T)),
        "wv": _chunked(np.ascontiguousarray(np.asarray(Wv_w, np.float32).T)),
        "wo": _chunked(np.ascontiguousarray(np.asarray(Wo_w, np.float32).T)),
    }
    biases = {"bq": np.asarray(Wq_b, np.float32), "bk": np.asarray(Wk_b, np.float32),
              "bv": np.asarray(Wv_b, np.float32), "bo": np.asarray(Wo_b, np.float32)}
    has_bias = {nm: bool(np.any(b)) for nm, b in biases.items()}

    kT = [_chunked(np.ascontiguousarray(key[b].T)) for b in range(B)]
    vT = [_chunked(np.ascontiguousarray(value[b].T)) for b in range(B)]
    ident = np.eye(P, dtype=NP_BF16)

    in_maps = []
    for c in range(N_CORES):
        b, qc = c // (N_CORES // B), c % (N_CORES // B)
        qslice = query[b, qc * LQ:(qc + 1) * LQ, :]
        m = {
            "qT": _part_major(np.ascontiguousarray(qslice.T)),
            "kT": kT[b],
            "vT": vT[b],
            **w,
        }
        if AV_FLIP:
            m["ident"] = ident
        for nm, hb in has_bias.items():
            if hb:
                m[nm] = biases[nm].reshape(1, D)
        in_maps.append(m)
    return in_maps, has_bias


def gather_output(results):
    out = np.empty((B, L, D), dtype=np.float32)
    for c in range(N_CORES):
        b, qc = c // (N_CORES // B), c % (N_CORES // B)
        oT = results[c]["outT"].reshape(P, S, LQ).transpose(1, 0, 2).reshape(D, LQ)
        out[b, qc * LQ:(qc + 1) * LQ, :] = oT.T
    return out


def kernel(**inputs) -> np.ndarray:
    in_maps, has_bias = prepare_inputs(**inputs)
    nc = _get_program(has_bias)
    res = run_bass_kernel_spmd(nc, in_maps, list(range(N_CORES)))
    return gather_output(res.results)

